# revision 9
# baseline (speedup 1.0000x reference)
"""Trainium2 Bass kernel for CCSequenceModel (2-layer GRU encoder ->
autoregressive 2-layer GRU decoder with feedback).

Layout: per core B=512 batch, split into 2 chunks of 256. All on-chip
tensors are "chunk-stacked" (128, 256): partitions 0:64 = H dims for
batch chunk 0, partitions 64:128 = H dims for batch chunk 1. Free dim =
256 batch elements. Hidden-state tiles, gate tiles, psum tiles all share
this layout, so every elementwise op uses all 128 lanes.

Matmuls contract over H (or the input dim) per chunk via tile_position
row/col offsets {0, 64}. Weights are host-packed transposed and
duplicated at partition offsets 0 and 64.

fp32 matmuls self-load weights (S3_LW struct) and can carry at most one
sync-wait, so each step a PE nop "gate" observes all cross-engine producers
first (via add_dep_helper) and the matmuls are ordered behind it.
"""

import numpy as np

import concourse.bass as bass
import concourse.mybir as mybir
import concourse.tile as tile
from concourse.bass_utils import run_bass_kernel_spmd
B, T_IN, N_IN, H, T_OUT = 4096, 256, 4, 64, 180
NCORES = 8
BC = B // NCORES  # 512 batch per core
CH = BC // 2      # 256 batch per chunk (free dim of every tile)
FP = mybir.dt.float32
AF = mybir.ActivationFunctionType
ALU = mybir.AluOpType

ENC_GRP = 8   # encoder steps per x-DMA group
DEC_GRP = 6   # decoder steps per output-staging group

# weight slot order in the packed (NW, 128, 64) array
_WSLOTS = [
    "E0x_r", "E0x_z", "E0x_n", "E0h_r", "E0h_z", "E0h_n",
    "E1i_r", "E1i_z", "E1i_n", "E1h_r", "E1h_z", "E1h_n",
    "D0i_r", "D0i_z", "D0i_n", "D0h_r", "D0h_z", "D0h_n",
    "D1i_r", "D1i_z", "D1i_n", "D1h_r", "D1h_z", "D1h_n",
    "HD",
]
WIDX = {n: i for i, n in enumerate(_WSLOTS)}
NW = len(_WSLOTS)

# bias columns: per cell (E0,E1,D0,D1): [b_r, b_z, -b_z, bhh_n, bih_n]
_CELLS = ["E0", "E1", "D0", "D1"]
NBIAS = 4 * 5 + 1  # + head bias col
HEAD_B = 20


def _pack_weights(inp):
    """Pack all weights/biases into (NW,128,64) and (NBIAS,128) arrays."""
    wp = np.zeros((NW, 128, 64), np.float32)
    bp = np.zeros((NBIAS, 128), np.float32)

    def put_w(name, m):  # m: (K, M) pre-transposed lhsT
        k, mm = m.shape
        wp[WIDX[name], 0:k, 0:mm] = m
        wp[WIDX[name], 64:64 + k, 0:mm] = m

    def gates(w):  # (3H, in) -> per-gate transposed (in, 64)
        return [np.ascontiguousarray(w[g * H:(g + 1) * H].T) for g in range(3)]

    for pre, wih, whh in [
        ("E0", inp["enc_Wih0"], inp["enc_Whh0"]),
        ("E1", inp["enc_Wih1"], inp["enc_Whh1"]),
        ("D0", inp["dec_Wih0"], inp["dec_Whh0"]),
        ("D1", inp["dec_Wih1"], inp["dec_Whh1"]),
    ]:
        gi, gh = gates(np.asarray(wih)), gates(np.asarray(whh))
        xi = "x" if pre == "E0" else "i"
        for g, nm in enumerate("rzn"):
            put_w(f"{pre}{xi}_{nm}", gi[g])
            put_w(f"{pre}h_{nm}", gh[g])

    hd = np.zeros((H, 64), np.float32)
    hd[:, 0] = np.asarray(inp["Wcv"])[0]
    hd[:, 1] = np.asarray(inp["Won"])[0]
    put_w("HD", hd)

    def put_b(col, v):  # v: (64,)
        bp[col, 0:64] = v
        bp[col, 64:128] = v

    lut = {"E0": ("enc_bih0", "enc_bhh0"), "E1": ("enc_bih1", "enc_bhh1"),
           "D0": ("dec_bih0", "dec_bhh0"), "D1": ("dec_bih1", "dec_bhh1")}
    for ci, pre in enumerate(_CELLS):
        bih, bhh = np.asarray(inp[lut[pre][0]]), np.asarray(inp[lut[pre][1]])
        base = ci * 5
        put_b(base + 0, bih[0:H] + bhh[0:H])
        put_b(base + 1, bih[H:2 * H] + bhh[H:2 * H])
        put_b(base + 2, -(bih[H:2 * H] + bhh[H:2 * H]))
        put_b(base + 3, bhh[2 * H:3 * H])
        put_b(base + 4, bih[2 * H:3 * H])
    # head bias: partition 0/64 = bcv, 1/65 = bon
    bp[HEAD_B, 0] = np.asarray(inp["bcv"])[0]
    bp[HEAD_B, 1] = np.asarray(inp["bon"])[0]
    bp[HEAD_B, 64] = np.asarray(inp["bcv"])[0]
    bp[HEAD_B, 65] = np.asarray(inp["bon"])[0]
    return wp, bp


def build_nc(t_in=T_IN, t_out=T_OUT):
    assert t_in % ENC_GRP == 0 and t_out % DEC_GRP == 0
    nc = bass.Bass()
    xt_d = nc.dram_tensor("xt", [t_in, 2, N_IN, CH], FP, kind="ExternalInput")
    wp_d = nc.dram_tensor("wp", [NW, 128, 64], FP, kind="ExternalInput")
    bp_d = nc.dram_tensor("bp", [NBIAS, 128], FP, kind="ExternalInput")
    out_d = nc.dram_tensor("out", [2, t_out, 2, CH], FP, kind="ExternalOutput")

    with tile.TileContext(nc) as tc:
        with (
            tc.tile_pool(name="const", bufs=1) as cpool,
            tc.tile_pool(name="state", bufs=1) as spool,
            tc.tile_pool(name="xin", bufs=3) as xpool,
            tc.tile_pool(name="gates", bufs=3) as gpool,
            tc.tile_pool(name="stage", bufs=2) as stpool,
            tc.tile_pool(name="ps", bufs=8, space="PSUM") as pspool,
        ):
            wt = cpool.tile([128, NW * 64], FP)
            d_w = nc.sync.dma_start(
                wt.rearrange("p (n f) -> p n f", n=NW),
                wp_d.rearrange("n p f -> p n f"),
            )
            bt = cpool.tile([128, NBIAS], FP)
            d_b = nc.sync.dma_start(bt[:], bp_d.rearrange("n p -> p n"))

            h1 = spool.tile([128, CH], FP)
            h2 = spool.tile([128, CH], FP)
            zrow = spool.tile([128, CH], FP)
            m1 = nc.vector.memset(h1[:], 0.0)
            m2 = nc.vector.memset(h2[:], 0.0)
            m3 = nc.vector.memset(zrow[:], 0.0)

            del d_w, d_b, m1, m2, m3

            def w_ap(name, c, k):
                s = WIDX[name] * 64
                return wt[c * 64:c * 64 + k, s:s + 64]

            def b_ap(cell, j):
                col = _CELLS.index(cell) * 5 + j
                return bt[:, col:col + 1]

            def mm_region(out_ap, contribs, c):
                """contribs: list of (lhsT_name, K, rhs_ap). Emitted in order;
                first start=True, last stop=True, accumulate in between."""
                n = len(contribs)
                for i, (wn, k, rhs) in enumerate(contribs):
                    nc.tensor.matmul(
                        out_ap, w_ap(wn, c, k), rhs,
                        start=(i == 0), stop=(i == n - 1),
                        tile_position=(c * 64, out_ap.base_partition()),
                    )

            def gru_cell(cell, in_contribs, h, make_azh=False):
                """in_contribs(c, gate_suffix) -> list of (wname, K, rhs_ap)
                for the input part; the h-part is prepended here.
                Updates h in place. Returns (a, zh) if make_azh."""
                pre = cell
                ps_rz = pspool.tile([128, 512], FP, tag="ps")
                ps_n = pspool.tile([128, 512], FP, tag="ps")
                for c in (0, 1):
                    hr = h[c * 64:(c + 1) * 64, :]
                    # r / z: h-part first (ready at step start), input last
                    mm_region(ps_rz[c * 64:(c + 1) * 64, 0:CH],
                              [(f"{pre}h_r", H, hr)] + in_contribs(c, "r"), c)
                    mm_region(ps_rz[c * 64:(c + 1) * 64, CH:2 * CH],
                              [(f"{pre}h_z", H, hr)] + in_contribs(c, "z"), c)
                    mm_region(ps_n[c * 64:(c + 1) * 64, 0:CH],
                              [(f"{pre}h_n", H, hr)], c)
                    mm_region(ps_n[c * 64:(c + 1) * 64, CH:2 * CH],
                              in_contribs(c, "n"), c)
                r = gpool.tile([128, CH], FP, tag="r")
                z = gpool.tile([128, CH], FP, tag="z")
                z1m = gpool.tile([128, CH], FP, tag="z1m")
                nc.scalar.activation(r[:], ps_rz[:, 0:CH], AF.Sigmoid,
                                     bias=b_ap(cell, 0))
                nc.scalar.activation(z[:], ps_rz[:, CH:2 * CH], AF.Sigmoid,
                                     bias=b_ap(cell, 1))
                zh = gpool.tile([128, CH], FP, tag="zh")
                nc.gpsimd.tensor_mul(zh[:], z[:], h[:])
                nc.scalar.activation(z1m[:], ps_rz[:, CH:2 * CH], AF.Sigmoid,
                                     bias=b_ap(cell, 2), scale=-1.0)
                tmp = gpool.tile([128, CH], FP, tag="tmp")
                # tmp = (ghn + bhh_n) * r
                nc.vector.scalar_tensor_tensor(
                    tmp[:], ps_n[:, 0:CH], b_ap(cell, 3), r[:],
                    op0=ALU.add, op1=ALU.mult)
                npre = gpool.tile([128, CH], FP, tag="npre")
                nc.vector.tensor_add(npre[:], tmp[:], ps_n[:, CH:2 * CH])
                n_t = gpool.tile([128, CH], FP, tag="n")
                nc.scalar.activation(n_t[:], npre[:], AF.Tanh,
                                     bias=b_ap(cell, 4))
                a = gpool.tile([128, CH], FP, tag="a")
                nc.vector.tensor_mul(a[:], z1m[:], n_t[:])
                nc.vector.tensor_add(h[:], a[:], zh[:])
                if make_azh:
                    return a, zh
                return None

            # ---------------- encoder ----------------
            for g in range(t_in // ENC_GRP):
                xt_t = xpool.tile([128, ENC_GRP * CH], FP, tag="xt")
                src = xt_d[g * ENC_GRP:(g + 1) * ENC_GRP]
                for c in (0, 1):
                    nc.sync.dma_start(
                        xt_t[c * 64:c * 64 + N_IN, :].rearrange(
                            "p (t b) -> p t b", t=ENC_GRP),
                        src[:, c].rearrange("t f b -> f t b"),
                    )
                for s in range(ENC_GRP):
                    off = s * CH

                    def e0_in(c, gs, _off=off, _xt=xt_t):
                        return [(f"E0x_{gs}", N_IN,
                                 _xt[c * 64:c * 64 + N_IN, _off:_off + CH])]

                    gru_cell("E0", e0_in, h1)

                    def e1_in(c, gs, _h1=h1):
                        return [(f"E1i_{gs}", H, _h1[c * 64:(c + 1) * 64, :])]

                    gru_cell("E1", e1_in, h2)

            # ---------------- decoder ----------------
            prev_t, prev_off = zrow, 0
            for g in range(t_out // DEC_GRP):
                stage = stpool.tile([128, DEC_GRP * CH], FP, tag="stage")
                for s in range(DEC_GRP):
                    off = s * CH

                    def d0_in(c, gs, _p=prev_t, _o=prev_off):
                        return [("D0i_" + gs, 1,
                                 _p[c * 64:c * 64 + 1, _o:_o + CH])]

                    a1, zh1 = gru_cell("D0", d0_in, h1, make_azh=True)

                    # d2 input = h1_new = a1 + zh1, split into two matmuls
                    def d1_in(c, gs, _a=a1, _z=zh1):
                        return [("D1i_" + gs, H, _z[c * 64:(c + 1) * 64, :]),
                                ("D1i_" + gs, H, _a[c * 64:(c + 1) * 64, :])]

                    a2, zh2 = gru_cell("D1", d1_in, h2, make_azh=True)

                    # heads: [cv; logit] = HD.T @ (zh2 + a2)
                    ps_h = pspool.tile([128, 512], FP, tag="ps")
                    for c in (0, 1):
                        ha = ps_h[c * 64:c * 64 + 2, 0:CH]
                        nc.tensor.matmul(
                            ha, w_ap("HD", c, H)[:, 0:2],
                            zh2[c * 64:(c + 1) * 64, :],
                            start=True, stop=False,
                            tile_position=(c * 64, c * 64))
                        nc.tensor.matmul(
                            ha, w_ap("HD", c, H)[:, 0:2],
                            a2[c * 64:(c + 1) * 64, :],
                            start=False, stop=True,
                            tile_position=(c * 64, c * 64))
                        nc.scalar.activation(
                            stage[c * 64:c * 64 + 2, off:off + CH],
                            ha, AF.Identity,
                            bias=bt[c * 64:c * 64 + 2, HEAD_B:HEAD_B + 1])
                    prev_t, prev_off = stage, off
                for c in (0, 1):
                    nc.sync.dma_start(
                        out_d[c, g * DEC_GRP:(g + 1) * DEC_GRP].rearrange(
                            "t p b -> p t b"),
                        stage[c * 64:c * 64 + 2, :].rearrange(
                            "p (t b) -> p t b", t=DEC_GRP),
                    )
    _split_mm_waits(nc)
    return nc


SPLIT_TYPES = {
    "InstMatmult", "InstActivation", "InstTensorTensor",
    "InstTensorScalarPtr", "InstMemset", "InstTensorCopy",
    "InstCustomDveAnt", "InstTensorReduce", "InstDMACopy", "InstNoOp",
    "InstDrain", "InstEventSemaphore",
}


def _split_mm_waits(nc):
    """TRN2 engine instructions support very few sync waits (the fp32
    self-loading matmul S3_LW struct, ACT S3D3_AC, etc. reject >1).
    Keep one wait per instruction and hoist the rest onto injected
    same-engine nops placed immediately before it."""
    for f in nc.m.functions:
        for blk in f.blocks:
            new = []
            k = 0
            for inst in blk.instructions:
                si = inst.sync_info
                if (type(inst).__name__ in SPLIT_TYPES and si is not None
                        and si.on_wait and len(si.on_wait) > 1):
                    waits = list(si.on_wait)
                    for w in waits[1:]:
                        nop = mybir.InstNoOp(
                            name=f"{inst.name}-wsplit{k}", ins=[], outs=[])
                        k += 1
                        nop.engine = inst.engine
                        nop.sync_info = mybir.SyncInfo(
                            on_wait=[w], on_update=[])
                        new.append(nop)
                    inst.sync_info = mybir.SyncInfo(
                        on_wait=waits[:1], on_update=list(si.on_update or []))
                new.append(inst)
            blk.instructions[:] = new
    return nc


_CACHE = {}


def _get_nc(t_in=T_IN, t_out=T_OUT):
    key = (t_in, t_out)
    if key not in _CACHE:
        _CACHE[key] = build_nc(t_in, t_out)
    return _CACHE[key]


def make_in_maps(inputs, t_in=T_IN):
    x = np.asarray(inputs["x"], dtype=np.float32)
    wp, bp = _pack_weights(inputs)
    in_maps = []
    for i in range(NCORES):
        xc = x[i * BC:(i + 1) * BC, :t_in]  # (512, t_in, 4)
        xt = np.ascontiguousarray(
            xc.reshape(2, CH, t_in, N_IN).transpose(2, 0, 3, 1))
        in_maps.append({"xt": xt, "wp": wp, "bp": bp})
    return in_maps


def unpack_outputs(results, t_out=T_OUT):
    outs = np.stack([r["out"] for r in results])  # (8, 2, t_out, 2, 256)
    arr = outs.transpose(0, 1, 4, 2, 3).reshape(B, t_out, 2)
    cvs = np.ascontiguousarray(arr[..., 0:1])
    logits = np.ascontiguousarray(arr[..., 1:2])
    return logits, cvs


def kernel(**inputs):
    nc = _get_nc()
    in_maps = make_in_maps(inputs)
    res = run_bass_kernel_spmd(nc, in_maps, list(range(NCORES)))
    return unpack_outputs(res.results)
